# revision 39
# baseline (speedup 1.0000x reference)
"""Trainium2 Bass kernel for nn_Logic_Learning_Model (declarative logic-rule
point-process log-likelihood).

Algorithm (factorized, validated vs reference at ~4e-7 rel err in numpy):
For each sample, all features are masked weighted sums over per-predicate
event arrays evaluated at 512 query times (255 head-event times + 1 pad +
256 grid points):

  feat0(q) = e^{2(Ck-tq)} * sum_j [tq > t1_j+TOL] * g_j(Ck)
             g_j(Ck) = [s1_j==1] * e^{2(t1_j-Ck)} * What_j
             What_j  = e^{C2-t1_j} * sum_i [t0_i < t1_j-TOL][s0_i==1] e^{t0_i-C2}
  feat1(q) = e^{Ck-tq} * sum_j [tq > t2_j+TOL] * [s2_j==1] e^{t2_j-Ck}
  feat2(q) = e^{Ck-tq} * ( D'(q) - C'(q) ),  D' = sum [tq>=t3] v3,
             C' = sum [tq > t3+TOL] v3,  v3_j = [s3_j==0] e^{t3_j-Ck}
  sh[idx(q)] = sum_j [tq > th_j] * (sh_j - sh_{j-1,wrap}) + sh_255

Ck is a per-query-block shift (C1=38.4 for tq<38.4, C2=76.8 otherwise) to
keep every exponential inside fp32 range; both variants are computed and
selected per query.  Masks are 0/1 bf16 tiles built by single tensor_scalar
compares on the vector engine over an int16-quantized time domain
(q(x) = rint(x*851 - 32768), all quantization on the host with one rounding
function so exact f32 ties stay ties) -- the 16-bit input unlocks the DVE
2x packed mode.  Weighted sums run on the PE as bf16 matmuls with
Dekker-split (hi+lo) weight vectors accumulating in fp32 PSUM.

PSUM row groups per sample (single bank): A+E@0-4 (E on row 4 via
zero-padded gA/dsh lhsT columns; E covers only the 256 grid queries --
head-query E is just sh_q, supplied from the host), B@32-35, and
band=D'-C'@64-67 (the subtract happens in PSUM via +vC@mD then -vC@mC).
Per-sample results are copied (ACT engine) into a persistent
[128, 32*512] staging tile; per 8-sample block, three row-group DMAs
evacuate to a DRAM staging buffer and four strided gather DMAs (on the
ACT hwdge queue, keeping the sync queue free for query broadcasts)
rebuild the [128 (qt,s), 15 roles x 128 q] layout for the batched
post-processing phase.  Phase 1/2 (the feat0 inner sums) run in 8-sample
blocks so phase-3 A-matmuls only wait on their own block, avoiding
head-of-line blocking of the in-order PE queue.  All event-major inputs
are pre-transposed on the host (contiguous DMAs).

Sharding: pure data parallel, 32 samples per core on 8 cores; each core
returns 128 per-(sample,query-tile) partial sums; host adds them up.
"""

import numpy as np

import concourse.bass as bass
import concourse.mybir as mybir
from concourse.tile import TileContext

F32 = mybir.dt.float32
BF16 = mybir.dt.bfloat16
I32 = mybir.dt.int32
I16 = mybir.dt.int16
U8 = mybir.dt.uint8

# int16 compare-domain quantization: q(x) = rint(x*QSCL + QOFF).  851*76.8
# = 65357 so the full time range spans the int16 range; resolution 1.2e-3.
# All quantization happens host-side with one rounding function so exact
# f32 ties (tq == th_j) stay exact ties in int16.
QSCL = 851.0
QOFF = -32768.0

NCORES = 8
S = 32          # samples per core
E = 256         # events per predicate
EH = 128        # half (one partition tile)
Q = 512         # padded query count: 255 head + 1 pad + 256 grid
T_MAX = 76.8
RES = 0.3
TOL = 0.1
C1 = 38.4
C2 = 76.8

AX = mybir.AxisListType
OP = mybir.AluOpType
ACTF = mybir.ActivationFunctionType


def bcast(ap, n=128):
    """0-stride partition broadcast view of a flat DRAM AP."""
    return bass.AP(ap.tensor, ap.offset, [[0, n]] + list(ap.ap))


def build_nc():
    from concourse.bacc import Bacc
    nc = Bacc(None, target_bir_lowering=False)
    times_d = nc.dram_tensor("times", [S, 5, E], F32, kind="ExternalInput")
    # event-major (pre-transposed on host) copies: contiguous DMAs instead
    # of 4-byte-strided transposing loads (~2.6us -> ~0.3us each)
    timesT_d = nc.dram_tensor("timesT", [5, 2, EH, S], F32, kind="ExternalInput")
    statesT_d = nc.dram_tensor("statesT", [5, 2, EH, S], I32, kind="ExternalInput")
    base_d = nc.dram_tensor("base", [1], F32, kind="ExternalInput")
    weights_d = nc.dram_tensor("weights", [3], F32, kind="ExternalInput")
    # per-sample query vectors: [head th[1:256] | pad=th[255] | grid], int16
    queries_d = nc.dram_tensor("queries", [S, Q], I16, kind="ExternalInput")
    # mask thresholds: int16-quantized values stored as f32 (the DVE
    # per-partition scalar operand must be float32).
    # kinds: 0=t1+TOL 1=t2+TOL 2=t3+TOL 3=t3 4=t4 5=t0
    thr_d = nc.dram_tensor("thr", [6, 2, EH, S], F32, kind="ExternalInput")
    # int16-quantized fl(t1 - TOL) rows for the phase-1 masks
    t1m16_d = nc.dram_tensor("t1m16", [S, E], I16, kind="ExternalInput")
    # host-computed dsh = sh_j - sh_{j-1,wrap} and escol (head rows: 1.0,
    # grid rows: 1-2*sh[255])
    dshT_d = nc.dram_tensor("dshT", [2, EH, S], F32, kind="ExternalInput")
    escol_d = nc.dram_tensor("escol", [128], F32, kind="ExternalInput")
    # head-query E values sh_q, [ (qt0|qt1) x 32 samples, 128 q ] f32
    shq_d = nc.dram_tensor("shq", [64, 128], F32, kind="ExternalInput")
    # grid rows pre-replicated for the post-phase query matrix (constant)
    gridq_d = nc.dram_tensor("gridq", [2, S, EH], F32, kind="ExternalInput")
    # consts[:, 0] = qtmask (1 for head rows), consts[:, 1] = pad column mask
    consts_d = nc.dram_tensor("consts", [128, 2], F32, kind="ExternalInput")
    staging_d = nc.dram_tensor("staging", [15, S, Q], F32, kind="Internal")
    out_d = nc.dram_tensor("out", [128], F32, kind="ExternalOutput")

    with TileContext(nc) as tc:
        _build(tc, nc, times_d, timesT_d, statesT_d, base_d, weights_d,
               queries_d, thr_d, t1m16_d, dshT_d, escol_d, shq_d, gridq_d,
               consts_d, staging_d, out_d)
    nc.finalize()
    return nc


def _build(tc, nc, times_d, timesT_d, statesT_d, base_d, weights_d,
           queries_d, thr_d, t1m16_d, dshT_d, escol_d, shq_d, gridq_d,
           consts_d, staging_d, out_d):
    cp = tc.alloc_tile_pool(name="const", bufs=1)
    sp = tc.alloc_tile_pool(name="samp", bufs=3)
    qp = tc.alloc_tile_pool(name="qbc", bufs=4)
    mp = tc.alloc_tile_pool(name="mask", bufs=6)
    pp = tc.alloc_tile_pool(name="psum", bufs=1, space="PSUM")
    pw = tc.alloc_tile_pool(name="psumw", bufs=2, space="PSUM")

    # prefetch the first two query broadcast blocks ahead of all other
    # input loads: the phase-3 masks (vector engine, the critical path)
    # depend only on these plus the small threshold tiles
    tq16_pre = []
    for blk4 in range(2):
        tqb = qp.tile([128, 4 * Q], I16, tag="tq16", name="t")
        qsrc = bass.AP(queries_d[:].tensor,
                       queries_d[:].offset + blk4 * 4 * Q,
                       [[0, 128], [1, 4 * Q]])
        nc.sync.dma_start(out=tqb[:], in_=qsrc)
        tq16_pre.append(tqb)

    # ---------------- phase 0: load events + batched prep ----------------
    # per-(array, half) event/state tiles, [128 events, 32 samples]
    T = {}
    ST = {}
    for a in range(5):
        for kt in range(2):
            t_t = cp.tile([EH, S], F32, tag=f"T{a}{kt}", name="t")
            s_t = cp.tile([EH, S], I32, tag=f"S{a}{kt}", name="t")
            nc.sync.dma_start(out=t_t[:], in_=timesT_d[a, kt])
            nc.scalar.dma_start(out=s_t[:], in_=statesT_d[a, kt])
            T[a, kt] = t_t
            ST[a, kt] = s_t

    # int16 mask-threshold tiles [128 events, 32 samples]
    thr = {}
    for k in range(6):
        for kt in range(2):
            th_t = cp.tile([EH, S], F32, tag=f"thr{k}{kt}", name="t")
            (nc.sync if (k + kt) % 2 else nc.scalar).dma_start(
                out=th_t[:], in_=thr_d[k, kt])
            thr[k, kt] = th_t

    # base/weights broadcast columns (0-stride DMA from DRAM)
    wbbc = cp.tile([128, 4], F32, tag="wbbc", name="t")
    nc.vector.memset(wbbc[:], 0.0)
    nc.sync.dma_start(out=wbbc[:, 0:3], in_=bcast(weights_d[:]))
    nc.sync.dma_start(out=wbbc[:, 3:4], in_=bcast(base_d[:]))
    negw2 = cp.tile([128, 1], F32, tag="negw2", name="t")
    nc.vector.tensor_scalar(out=negw2[:], in0=wbbc[:, 2:3], scalar1=-1.0,
                            scalar2=None, op0=OP.mult)

    # consts: col0 = query-type mask (1.0 head rows), col1 = pad-column mask
    consts = cp.tile([128, 2], F32, tag="consts", name="t")
    nc.sync.dma_start(out=consts[:], in_=consts_d[:])
    qtmask = consts[:, 0:1]
    padcol = consts[:, 1:2]

    # ---- batched exponentials / state masks / weight vectors per half ----
    ew = {}     # exp tiles keyed by (name, kt)
    sm = {}
    for kt in range(2):
        # exp args -> one tile per needed exponential, [128, 32]
        def _exp(tag, src, scale, off):
            arg = sp.tile([EH, S], F32, tag=f"arg{tag}{kt}", name="t")
            nc.vector.tensor_scalar(out=arg[:], in0=src[:], scalar1=scale,
                                    scalar2=off, op0=OP.mult, op1=OP.add)
            e_t = cp.tile([EH, S], F32, tag=f"e{tag}{kt}", name="t")
            nc.scalar.activation(e_t[:], arg[:], ACTF.Exp)
            return e_t

        ew["w0", kt] = _exp("w0", T[0, kt], 1.0, -C2)       # e^{t0-C2}
        ew["c2t1", kt] = _exp("c2t1", T[1, kt], -1.0, C2)   # e^{C2-t1}
        ew["g1", kt] = _exp("g1", T[1, kt], 2.0, -2.0 * C1)  # e^{2(t1-C1)}
        ew["g2", kt] = _exp("g2", T[1, kt], 2.0, -2.0 * C2)
        ew["v21", kt] = _exp("v21", T[2, kt], 1.0, -C1)
        ew["v22", kt] = _exp("v22", T[2, kt], 1.0, -C2)
        ew["v31", kt] = _exp("v31", T[3, kt], 1.0, -C1)
        ew["v32", kt] = _exp("v32", T[3, kt], 1.0, -C2)

        for a, val, tag in ((0, 1, "s0"), (1, 1, "s1"), (2, 1, "s2"), (3, 0, "s3")):
            m = cp.tile([EH, S], F32, tag=f"{tag}{kt}", name="t")
            nc.vector.tensor_scalar(out=m[:], in0=ST[a, kt][:], scalar1=val,
                                    scalar2=None, op0=OP.is_equal)
            sm[tag, kt] = m

        # [t3 <= C1]: zeroes v3C1 entries that no blk1 query can ever select;
        # keeps sum(v3C1) bounded so D'/C' stay in fp32 range.
        m31 = cp.tile([EH, S], F32, tag=f"m31{kt}", name="t")
        nc.vector.tensor_scalar(out=m31[:], in0=T[3, kt][:], scalar1=C1,
                                scalar2=None, op0=OP.is_le)
        sm["m31", kt] = m31

    def dekker(dst, blk0, src32, tmp_tag):
        """write bf16 (hi, lo) blocks of src32 [128, S] into dst block cols
        [blk0*S:(blk0+1)*S] and [(blk0+1)*S:(blk0+2)*S]"""
        hi = dst[:, blk0 * S:(blk0 + 1) * S]
        lo = dst[:, (blk0 + 1) * S:(blk0 + 2) * S]
        nc.vector.tensor_copy(out=hi, in_=src32[:])
        tmp = sp.tile([EH, S], F32, tag=tmp_tag, name="t")
        nc.vector.tensor_copy(out=tmp[:], in_=hi)
        nc.vector.tensor_tensor(out=lo, in0=src32[:], in1=tmp[:],
                                op=OP.subtract)

    # w0 pairs (feat0 inner sum weights), [128, 2*S]: cols 2s,2s+1 = h,l
    w0pair = {}
    for kt in range(2):
        w0 = sp.tile([EH, S], F32, tag=f"w0m{kt}", name="t")
        nc.vector.tensor_tensor(out=w0[:], in0=ew["w0", kt][:], in1=sm["s0", kt][:],
                                op=OP.mult)
        pair = cp.tile([EH, 2 * S], BF16, tag=f"w0pair{kt}", name="t")
        dekker(pair, 0, w0, f"w0tmp{kt}")
        w0pair[kt] = pair

    # v2 / v3 quads [128, 4*S]: cols 4s..4s+3 = [vC1h vC1l vC2h vC2l]
    vB = {}
    vC = {}
    vCn = {}
    for kt in range(2):
        q_b = cp.tile([EH, 4 * S], BF16, tag=f"vB{kt}", name="t")
        q_c = cp.tile([EH, 4 * S], BF16, tag=f"vC{kt}", name="t")
        for ver, (e2tag, e3tag) in enumerate((("v21", "v31"), ("v22", "v32"))):
            v2 = sp.tile([EH, S], F32, tag=f"v2m{kt}{ver}", name="t")
            nc.vector.tensor_tensor(out=v2[:], in0=ew[e2tag, kt][:],
                                    in1=sm["s2", kt][:], op=OP.mult)
            dekker(q_b, 2 * ver, v2, f"dkb{kt}{ver}")
            v3 = sp.tile([EH, S], F32, tag=f"v3m{kt}{ver}", name="t")
            nc.vector.tensor_tensor(out=v3[:], in0=ew[e3tag, kt][:],
                                    in1=sm["s3", kt][:], op=OP.mult)
            if ver == 0:
                nc.vector.tensor_tensor(out=v3[:], in0=v3[:],
                                        in1=sm["m31", kt][:], op=OP.mult)
            dekker(q_c, 2 * ver, v3, f"dkc{kt}{ver}")
        # negated vC for the PSUM-side band subtract (D' - C')
        q_cn = cp.tile([EH, 4 * S], BF16, tag=f"vCn{kt}", name="t")
        nc.vector.tensor_scalar(out=q_cn[:], in0=q_c[:], scalar1=-1.0,
                                scalar2=None, op0=OP.mult)
        vB[kt] = q_b
        vC[kt] = q_c
        vCn[kt] = q_cn

    # dsh (bf16): sh_j - sh_{j-1 (wrap)} from host; stored zero-padded
    # [z z z z dsh] per sample so the E matmul lands on psum row 4.
    dsh = {}
    for kt in range(2):
        d32 = sp.tile([EH, S], F32, tag=f"dsh32{kt}", name="t")
        nc.sync.dma_start(out=d32[:], in_=dshT_d[kt])
        d = cp.tile([EH, 5 * S], BF16, tag=f"dsh{kt}", name="t")
        nc.vector.memset(d[:], 0.0)
        nc.vector.tensor_copy(out=d[:, 4 * S:5 * S], in_=d32[:])
        dsh[kt] = d

    # escol = 1 - 2*sh[255], per (sample,qt) partition row (host-computed)
    escol = cp.tile([128, 1], F32, tag="escol", name="t")
    nc.sync.dma_start(out=escol[:], in_=escol_d[:])

    # ------------- phase 1+2: per-sample What + g-vector assembly ----------
    # Processed in 8-sample blocks so phase-3 A-matmuls for block b only
    # depend on phase 1 of block b (avoids head-of-line blocking of the
    # in-order PE queue behind all 32 samples of phase 1).
    wst = cp.tile([128, 4 * S], F32, tag="wst", name="t")  # cols 4s.. = [j0h j0l j1h j1l]
    gA = {}
    for kt in range(2):
        # 5-col layout [g1h g1l g2h g2l 0]: the zero 5th column lets the A
        # matmul share psum rows 0-4 with the E matmul (E lands on row 4).
        g_t = cp.tile([EH, 5 * S], BF16, tag=f"gA{kt}", name="t")
        nc.vector.memset(g_t[:], 0.0)
        gA[kt] = g_t

    SB = 8  # phase-1/2 block size
    for blk in range(S // SB):
        for s in range(blk * SB, (blk + 1) * SB):
            t1bc = sp.tile([128, E], I16, tag="t1bc", name="t")
            nc.sync.dma_start(out=t1bc[:], in_=bcast(t1m16_d[s]))

            psw = pw.tile([128, 4], F32, tag="psw", name="t")
            mwts = []
            for ikt in range(2):
                mwt = sp.tile([128, E], BF16, tag=f"mwt{ikt}", name="t")
                nc.vector.tensor_scalar(out=mwt[:], in0=t1bc[:],
                                        scalar1=thr[5, ikt][:, s:s + 1],
                                        scalar2=None, op0=OP.is_gt)
                mwts.append(mwt)
            for jkt in range(2):
                for ikt in range(2):
                    nc.tensor.matmul(psw[:, 2 * jkt:2 * jkt + 2],
                                     mwts[ikt][:, jkt * EH:(jkt + 1) * EH],
                                     w0pair[ikt][:, s::S][:, 0:2],
                                     start=(ikt == 0), stop=(ikt == 1))
            nc.vector.tensor_copy(out=wst[:, s::S][:, 0:4], in_=psw[:])

        # phase 2 for this block: assemble gA columns blk*SB..(blk+1)*SB
        bs = slice(blk * SB, (blk + 1) * SB)
        for kt in range(2):
            wh = sp.tile([EH, SB], F32, tag=f"wh{kt}", name="t")
            # wst blocks: [j0h | j0l | j1h | j1l], each S wide
            nc.vector.tensor_tensor(
                out=wh[:], in0=wst[:, 2 * kt * S:(2 * kt + 1) * S][:, bs],
                in1=wst[:, (2 * kt + 1) * S:(2 * kt + 2) * S][:, bs],
                op=OP.add)
            nc.vector.tensor_tensor(out=wh[:], in0=wh[:],
                                    in1=ew["c2t1", kt][:, bs], op=OP.mult)
            for ver, etag in enumerate(("g1", "g2")):
                g32 = sp.tile([EH, SB], F32, tag=f"g32{kt}{ver}", name="t")
                nc.vector.tensor_tensor(out=g32[:], in0=ew[etag, kt][:, bs],
                                        in1=wh[:], op=OP.mult)
                nc.vector.tensor_tensor(out=g32[:], in0=g32[:],
                                        in1=sm["s1", kt][:, bs], op=OP.mult)
                hi = gA[kt][:, 2 * ver * S:(2 * ver + 1) * S][:, bs]
                lo = gA[kt][:, (2 * ver + 1) * S:(2 * ver + 2) * S][:, bs]
                nc.vector.tensor_copy(out=hi, in_=g32[:])
                tmp8 = sp.tile([EH, SB], F32, tag=f"dkg{kt}{ver}", name="t")
                nc.vector.tensor_copy(out=tmp8[:], in_=hi)
                nc.vector.tensor_tensor(out=lo, in0=g32[:], in1=tmp8[:],
                                        op=OP.subtract)

    # ------------- phase 3: per-sample masks + weighted sums ---------------
    # psum row groups (32-aligned, single bank): A+E@0-4 (E on row 4 via the
    # zero-padded gA/dsh lhsT columns), B@32-35, band(D'-C')@64-67.  The
    # band subtract happens in PSUM: +vC@mD then -vC@mC accumulate.  Masks
    # compare int16-quantized times so the DVE runs in its packed 16-bit
    # mode (~2x) instead of the 1x fp32 path.
    # stga32 holds all samples' evacuated results: free dim = (s, q).
    stga32 = cp.tile([128, S * Q], F32, tag="stga32", name="t")
    stage2 = cp.tile([128, 15 * 128], F32, tag="stage2", name="t")
    sd = staging_d[:]
    psums = []
    for i in range(3):
        t_ps = pp.tile([128, Q], F32, tag=f"pm{i}", name="t")
        nc.vector.memset(t_ps[:], 0.0)
        psums.append(t_ps)
    tq16big = None
    for s in range(S):
        if s % 4 == 0:
            if s < 8:
                tq16big = tq16_pre[s // 4]
            else:
                tq16big = qp.tile([128, 4 * Q], I16, tag="tq16", name="t")
                qsrc = bass.AP(queries_d[:].tensor,
                               queries_d[:].offset + s * Q,
                               [[0, 128], [1, 4 * Q]])
                nc.sync.dma_start(out=tq16big[:], in_=qsrc)
        tq16 = tq16big[:, (s % 4) * Q:(s % 4 + 1) * Q]
        psum = psums[s % 3]
        for kt in range(2):
            masks = []
            for kind, op in ((0, OP.is_gt), (1, OP.is_gt), (2, OP.is_gt),
                             (3, OP.is_ge)):
                m = mp.tile([128, Q], BF16, tag=f"m{kind}{kt}", name="t")
                nc.vector.tensor_scalar(out=m[:], in0=tq16,
                                        scalar1=thr[kind, kt][:, s:s + 1],
                                        scalar2=None, op0=op)
                masks.append(m)
            mA, mB, mC, mD = masks
            # E only needs the 256 grid queries; head-query E (= sh_q) is
            # overwritten into stage2 directly from the host shq input.
            mE = mp.tile([128, Q // 2], BF16, tag=f"m4{kt}", name="t")
            nc.vector.tensor_scalar(out=mE[:], in0=tq16[:, Q // 2:Q],
                                    scalar1=thr[4, kt][:, s:s + 1],
                                    scalar2=None, op0=OP.is_gt)
            st = (kt == 0)
            sp_ = (kt == 1)
            nc.tensor.matmul(psum[0:5, 0:Q], gA[kt][:, s::S][:, 0:5], mA[:],
                             start=st, stop=False)
            nc.tensor.matmul(psum[0:5, Q // 2:Q], dsh[kt][:, s::S][:, 0:5],
                             mE[:], start=False, stop=sp_)
            nc.tensor.matmul(psum[32:36, 0:Q], vB[kt][:, s::S][:, 0:4], mB[:],
                             start=st, stop=sp_)
            nc.tensor.matmul(psum[64:68, 0:Q], vC[kt][:, s::S][:, 0:4], mD[:],
                             start=st, stop=False)
            nc.tensor.matmul(psum[64:68, 0:Q], vCn[kt][:, s::S][:, 0:4], mC[:],
                             start=False, stop=sp_)
        # evacuate psum -> stga32 (ACT); A+E rows 0-4 and B rows 32-35 in
        # one copy (junk rows between), band in a second.
        nc.scalar.copy(stga32[0:36, s * Q:(s + 1) * Q], psum[0:36, 0:Q])
        nc.scalar.copy(stga32[64:68, s * Q:(s + 1) * Q], psum[64:68, 0:Q])
        evac = {7: (0, 8), 15: (8, 8), 23: (16, 8), 27: (24, 4), 31: (28, 4)}
        if s in evac:
            b0, bn = evac[s]
            src_v = stga32[:].rearrange("p (s q) -> p s q", s=S)
            for g in range(3):
                nc.scalar.dma_start(
                    out=staging_d[5 * g:5 * (g + 1), b0:b0 + bn, :],
                    in_=src_v[32 * g:32 * g + 5, b0:b0 + bn, :])
            # gather this block back: staging [15 r, bn s, 512 q] ->
            # stage2 rows {32 qt + b0 + s'}, (15 r, 128 q)
            for qt in range(4):
                src_g = bass.AP(sd.tensor, sd.offset + qt * 128 + b0 * Q,
                                [[Q, bn], [S * Q, 15], [1, 128]])
                nc.scalar.dma_start(
                    out=stage2[32 * qt + b0:32 * qt + b0 + bn,
                               :].rearrange("p (r q) -> p r q", r=15),
                    in_=src_g)

    # head-query E rows: overwrite role 4 of rows 0-63 with host sh_q
    nc.sync.dma_start(out=stage2[0:64, 4 * 128:5 * 128], in_=shq_d[:])

    # ------------- phase 4: batched post-processing ------------------------
    def R(r):
        return stage2[:, r * 128:(r + 1) * 128]

    # query matrix [128 (s,qt), 128]
    tq_m = cp.tile([128, 128], F32, tag="tqm", name="t")
    nc.vector.memset(tq_m[:], 0.0)
    nc.sync.dma_start(out=tq_m[0:32, :], in_=times_d[:, 4, 1:129])
    nc.sync.dma_start(out=tq_m[32:64, 0:127], in_=times_d[:, 4, 129:256])
    nc.sync.dma_start(out=tq_m[32:64, 127:128], in_=times_d[:, 4, 255:256])
    nc.sync.dma_start(out=tq_m[64:96, :], in_=gridq_d[0])
    nc.sync.dma_start(out=tq_m[96:128, :], in_=gridq_d[1])

    def tmp(tag):
        return cp.tile([128, 128], F32, tag=tag, name="t")

    # pairwise hi+lo sums (in place into the hi slot)
    # roles: 0-3 A quads, 4 E, 5-8 B, 10-13 band=D'-C' (9, 14 junk)
    for r in (0, 2, 5, 7, 10, 12):
        nc.vector.tensor_tensor(out=R(r), in0=R(r), in1=R(r + 1), op=OP.add)
    A1, A2, B1, B2, Bd1, Bd2 = (R(r) for r in (0, 2, 5, 7, 10, 12))

    blk = cp.tile([128, 128], U8, tag="blk", name="t")
    nc.vector.tensor_scalar(out=blk[:], in0=tq_m[:], scalar1=C1, scalar2=None,
                            op0=OP.is_ge)
    biasC1 = cp.tile([128, 1], F32, tag="biasC1", name="t")
    nc.vector.memset(biasC1[:], C1)
    biasC2 = cp.tile([128, 1], F32, tag="biasC2", name="t")
    nc.vector.memset(biasC2[:], C2)
    e1 = tmp("e1")
    nc.scalar.activation(e1[:], tq_m[:], ACTF.Exp, bias=biasC1[:], scale=-1.0)
    e2 = tmp("e2")
    nc.scalar.activation(e2[:], tq_m[:], ACTF.Exp, bias=biasC2[:], scale=-1.0)

    def sel(tag, on_true, on_false):
        o = tmp(tag)
        nc.vector.select(o, blk[:], on_true, on_false)
        return o

    esel = sel("esel", e2[:], e1[:])
    Asel = sel("Asel", A2, A1)
    Bsel = sel("Bsel", B2, B1)
    Bdsel = sel("Bdsel", Bd2, Bd1)

    feat0 = tmp("feat0")
    nc.vector.tensor_tensor(out=feat0[:], in0=esel[:], in1=Asel[:], op=OP.mult)
    nc.vector.tensor_tensor(out=feat0[:], in0=feat0[:], in1=esel[:], op=OP.mult)
    feat1 = tmp("feat1")
    nc.vector.tensor_tensor(out=feat1[:], in0=esel[:], in1=Bsel[:], op=OP.mult)
    feat2 = tmp("feat2")
    nc.vector.tensor_tensor(out=feat2[:], in0=Bdsel[:], in1=esel[:], op=OP.mult)

    eff0 = tmp("eff0")
    nc.vector.tensor_scalar(out=eff0[:], in0=R(4), scalar1=-2.0, scalar2=escol[:],
                            op0=OP.mult, op1=OP.add)

    combo = tmp("combo")
    nc.vector.tensor_scalar(out=combo[:], in0=feat0[:], scalar1=wbbc[:, 0:1],
                            scalar2=None, op0=OP.mult)
    nc.vector.scalar_tensor_tensor(out=combo[:], in0=feat1[:], scalar=wbbc[:, 1:2],
                                   in1=combo[:], op0=OP.mult, op1=OP.add)
    nc.vector.scalar_tensor_tensor(out=combo[:], in0=feat2[:], scalar=negw2[:],
                                   in1=combo[:], op0=OP.mult, op1=OP.add)
    logits = tmp("logits")
    nc.vector.tensor_tensor(out=logits[:], in0=combo[:], in1=eff0[:], op=OP.mult)
    nc.vector.tensor_scalar(out=logits[:], in0=logits[:], scalar1=wbbc[:, 3:4],
                            scalar2=None, op0=OP.add)
    # zero the pad query (qt==1 rows, col 127) via the pad-column mask
    nc.vector.tensor_tensor(out=logits[:, 127:128], in0=logits[:, 127:128],
                            in1=padcol, op=OP.mult)

    hsum = cp.tile([128, 1], F32, tag="hsum", name="t")
    nc.vector.tensor_reduce(out=hsum[:], in_=logits[:], axis=AX.X, op=OP.add)
    expt = tmp("expt")
    intcol = cp.tile([128, 1], F32, tag="intcol", name="t")
    nc.scalar.activation(expt[:], logits[:], ACTF.Exp, accum_out=intcol[:])
    nc.vector.tensor_scalar(out=intcol[:], in0=intcol[:], scalar1=-RES,
                            scalar2=None, op0=OP.mult)
    qtmaski = cp.tile([128, 1], U8, tag="qtmaski", name="t")
    nc.vector.tensor_scalar(out=qtmaski[:], in0=qtmask, scalar1=0.5,
                            scalar2=None, op0=OP.is_ge)
    rowpart = cp.tile([128, 1], F32, tag="rowpart", name="t")
    nc.vector.select(rowpart[:], qtmaski[:], hsum[:], intcol[:])
    nc.sync.dma_start(out=out_d[:], in_=rowpart[:])

    for pool in (pw, pp, mp, qp, sp, cp):
        pool.release()


_NC_CACHE = []


def _get_nc():
    if not _NC_CACHE:
        _NC_CACHE.append(build_nc())
    return _NC_CACHE[0]


def _q16(x):
    """int16 compare-domain quantization (must be the single rounding
    function for every quantized value so exact f32 ties stay ties)."""
    return np.clip(np.rint(x.astype(np.float64) * QSCL + QOFF),
                   -32768, 32767).astype(np.int16)


def make_inputs_for_core(times, states, base, weights, core):
    grid = np.arange(0.0, T_MAX, RES, dtype=np.float32)
    gridq = np.stack([np.tile(grid[0:128], (S, 1)), np.tile(grid[128:256], (S, 1))])
    consts = np.ones((128, 2), np.float32)
    consts[64:128, 0] = 0.0   # qtmask: 0 for grid rows (qt 2,3 blocks)
    consts[32:64, 1] = 0.0    # pad-column mask: 0 for qt1 block
    sl = slice(core * S, (core + 1) * S)
    t = np.ascontiguousarray(times[sl]).astype(np.float32)
    st = np.ascontiguousarray(states[sl]).astype(np.int32)
    queries_f = np.concatenate(
        [t[:, 4, 1:256], t[:, 4, 255:256], np.tile(grid, (S, 1))],
        axis=1).astype(np.float32)
    tolf = np.float32(TOL)
    # [6, S, 2, EH] -> transpose to [6, 2, EH, S] (event-major)
    thr = np.stack([
        _q16((t[:, 1] + tolf).astype(np.float32)),
        _q16((t[:, 2] + tolf).astype(np.float32)),
        _q16((t[:, 3] + tolf).astype(np.float32)),
        _q16(t[:, 3]),
        _q16(t[:, 4]),
        _q16(t[:, 0]),
    ]).astype(np.float32).reshape(6, S, 2, EH).transpose(0, 2, 3, 1)
    timesT = t.reshape(S, 5, 2, EH).transpose(1, 2, 3, 0)
    statesT = st.reshape(S, 5, 2, EH).transpose(1, 2, 3, 0)
    sh = st[:, 4, :]
    dsh = (sh - np.roll(sh, 1, axis=1)).astype(np.float32)   # [S, E]
    dshT = dsh.reshape(S, 2, EH).transpose(1, 2, 0)
    es_g = (1 - 2 * sh[:, 255]).astype(np.float32)
    escol = np.concatenate([np.ones(64, np.float32), np.tile(es_g, 2)])
    t1m16 = _q16((t[:, 1] - tolf).astype(np.float32))
    return {
        "times": t,
        "timesT": np.ascontiguousarray(timesT),
        "statesT": np.ascontiguousarray(statesT),
        "base": np.asarray(base, np.float32),
        "weights": np.asarray(weights, np.float32),
        "queries": np.ascontiguousarray(_q16(queries_f)),
        "thr": np.ascontiguousarray(thr),
        "t1m16": np.ascontiguousarray(t1m16),
        "dshT": np.ascontiguousarray(dshT),
        "escol": escol,
        "shq": np.ascontiguousarray(
            np.concatenate([sh[:, 0:128], sh[:, 128:256]]).astype(np.float32)),
        "gridq": np.ascontiguousarray(gridq).astype(np.float32),
        "consts": consts,
    }


def kernel(times, states, base, weights):
    from concourse.bass_utils import run_bass_kernel_spmd

    times = np.asarray(times, np.float32)
    states = np.asarray(states, np.int32)
    nc = _get_nc()
    in_maps = [make_inputs_for_core(times, states, base, weights, c)
               for c in range(NCORES)]
    res = run_bass_kernel_spmd(nc, in_maps, list(range(NCORES)))
    parts = np.stack([np.asarray(res.results[c]["out"]) for c in range(NCORES)])
    total = np.sum(parts.astype(np.float32), dtype=np.float32)
    return np.array([total], dtype=np.float32)


def run_traced(times, states, base, weights):
    """Profiled run; returns HW exec time in ns (or None if tracing off)."""
    from concourse.bass_utils import run_bass_kernel_spmd

    times = np.asarray(times, np.float32)
    states = np.asarray(states, np.int32)
    nc = _get_nc()
    in_maps = [make_inputs_for_core(times, states, base, weights, c)
               for c in range(NCORES)]
    res = run_bass_kernel_spmd(nc, in_maps, list(range(NCORES)), trace=True)
    return res.exec_time_ns


# revision 40
# speedup vs baseline: 1.0588x; 1.0588x over previous
"""Trainium2 Bass kernel for nn_Logic_Learning_Model (declarative logic-rule
point-process log-likelihood).

Algorithm (factorized, validated vs reference at ~4e-7 rel err in numpy):
For each sample, all features are masked weighted sums over per-predicate
event arrays evaluated at 512 query times (255 head-event times + 1 pad +
256 grid points):

  feat0(q) = e^{2(Ck-tq)} * sum_j [tq > t1_j+TOL] * g_j(Ck)
             g_j(Ck) = [s1_j==1] * e^{2(t1_j-Ck)} * What_j
             What_j  = e^{C2-t1_j} * sum_i [t0_i < t1_j-TOL][s0_i==1] e^{t0_i-C2}
  feat1(q) = e^{Ck-tq} * sum_j [tq > t2_j+TOL] * [s2_j==1] e^{t2_j-Ck}
  feat2(q) = e^{Ck-tq} * ( D'(q) - C'(q) ),  D' = sum [tq>=t3] v3,
             C' = sum [tq > t3+TOL] v3,  v3_j = [s3_j==0] e^{t3_j-Ck}
  sh[idx(q)] = sum_j [tq > th_j] * (sh_j - sh_{j-1,wrap}) + sh_255

Ck is a per-query-block shift (C1=38.4 for tq<38.4, C2=76.8 otherwise) to
keep every exponential inside fp32 range; both variants are computed and
selected per query.  Masks are 0/1 bf16 tiles built by single tensor_scalar
compares on the vector engine over an int16-quantized time domain
(q(x) = rint(x*851 - 32768), all quantization on the host with one rounding
function so exact f32 ties stay ties) -- the 16-bit input unlocks the DVE
2x packed mode.  Weighted sums run on the PE as bf16 matmuls with
Dekker-split (hi+lo) weight vectors accumulating in fp32 PSUM.

PSUM row groups per sample (single bank): A+E@0-4 (E on row 4 via
zero-padded gA/dsh lhsT columns; E covers only the 256 grid queries --
head-query E is just sh_q, supplied from the host), B@32-35, and
band=D'-C'@64-67 (the subtract happens in PSUM via +vC@mD then -vC@mC).
Per-sample results are copied (ACT engine) into a persistent
[128, 32*512] staging tile; per 8-sample block, three row-group DMAs
evacuate to a DRAM staging buffer and four strided gather DMAs (on the
ACT hwdge queue, keeping the sync queue free for query broadcasts)
rebuild the [128 (qt,s), 15 roles x 128 q] layout for the batched
post-processing phase.  Phase 1/2 (the feat0 inner sums) run in 8-sample
blocks so phase-3 A-matmuls only wait on their own block, avoiding
head-of-line blocking of the in-order PE queue.  All event-major inputs
are pre-transposed on the host (contiguous DMAs).

Sharding: pure data parallel, 32 samples per core on 8 cores; each core
returns 128 per-(sample,query-tile) partial sums; host adds them up.
"""

import numpy as np

import concourse.bass as bass
import concourse.mybir as mybir
from concourse.tile import TileContext

F32 = mybir.dt.float32
BF16 = mybir.dt.bfloat16
I32 = mybir.dt.int32
I16 = mybir.dt.int16
U8 = mybir.dt.uint8

# int16 compare-domain quantization: q(x) = rint(x*QSCL + QOFF).  851*76.8
# = 65357 so the full time range spans the int16 range; resolution 1.2e-3.
# All quantization happens host-side with one rounding function so exact
# f32 ties (tq == th_j) stay exact ties in int16.
QSCL = 851.0
QOFF = -32768.0

NCORES = 8
S = 32          # samples per core
E = 256         # events per predicate
EH = 128        # half (one partition tile)
Q = 512         # padded query count: 255 head + 1 pad + 256 grid
T_MAX = 76.8
RES = 0.3
TOL = 0.1
C1 = 38.4
C2 = 76.8

AX = mybir.AxisListType
OP = mybir.AluOpType
ACTF = mybir.ActivationFunctionType


def bcast(ap, n=128):
    """0-stride partition broadcast view of a flat DRAM AP."""
    return bass.AP(ap.tensor, ap.offset, [[0, n]] + list(ap.ap))


def build_nc():
    from concourse.bacc import Bacc
    nc = Bacc(None, target_bir_lowering=False)
    times_d = nc.dram_tensor("times", [S, 5, E], F32, kind="ExternalInput")
    # event-major (pre-transposed on host) copies: contiguous DMAs instead
    # of 4-byte-strided transposing loads (~2.6us -> ~0.3us each)
    timesT_d = nc.dram_tensor("timesT", [5, 2, EH, S], F32, kind="ExternalInput")
    statesT_d = nc.dram_tensor("statesT", [5, 2, EH, S], I32, kind="ExternalInput")
    base_d = nc.dram_tensor("base", [1], F32, kind="ExternalInput")
    weights_d = nc.dram_tensor("weights", [3], F32, kind="ExternalInput")
    # per-sample query vectors: [head th[1:256] | pad=th[255] | grid], int16
    queries_d = nc.dram_tensor("queries", [S, Q], I16, kind="ExternalInput")
    # mask thresholds: int16-quantized values stored as f32 (the DVE
    # per-partition scalar operand must be float32).
    # kinds: 0=t1+TOL 1=t2+TOL 2=t3+TOL 3=t3 4=t4 5=t0
    thr_d = nc.dram_tensor("thr", [6, 2, EH, S], F32, kind="ExternalInput")
    # int16-quantized fl(t1 - TOL) rows for the phase-1 masks
    t1m16_d = nc.dram_tensor("t1m16", [S, E], I16, kind="ExternalInput")
    # host-computed dsh = sh_j - sh_{j-1,wrap} and escol (head rows: 1.0,
    # grid rows: 1-2*sh[255])
    dshT_d = nc.dram_tensor("dshT", [2, EH, S], F32, kind="ExternalInput")
    escol_d = nc.dram_tensor("escol", [128], F32, kind="ExternalInput")
    # head-query E values sh_q, [ (qt0|qt1) x 32 samples, 128 q ] f32
    shq_d = nc.dram_tensor("shq", [64, 128], F32, kind="ExternalInput")
    # grid rows pre-replicated for the post-phase query matrix (constant)
    gridq_d = nc.dram_tensor("gridq", [2, S, EH], F32, kind="ExternalInput")
    # consts[:, 0] = qtmask (1 for head rows), consts[:, 1] = pad column mask
    consts_d = nc.dram_tensor("consts", [128, 2], F32, kind="ExternalInput")
    staging_d = nc.dram_tensor("staging", [15, S, Q], F32, kind="Internal")
    out_d = nc.dram_tensor("out", [128], F32, kind="ExternalOutput")

    with TileContext(nc) as tc:
        _build(tc, nc, times_d, timesT_d, statesT_d, base_d, weights_d,
               queries_d, thr_d, t1m16_d, dshT_d, escol_d, shq_d, gridq_d,
               consts_d, staging_d, out_d)
    nc.finalize()
    return nc


def _build(tc, nc, times_d, timesT_d, statesT_d, base_d, weights_d,
           queries_d, thr_d, t1m16_d, dshT_d, escol_d, shq_d, gridq_d,
           consts_d, staging_d, out_d):
    cp = tc.alloc_tile_pool(name="const", bufs=1)
    sp = tc.alloc_tile_pool(name="samp", bufs=3)
    qp = tc.alloc_tile_pool(name="qbc", bufs=4)
    mp = tc.alloc_tile_pool(name="mask", bufs=3)
    pp = tc.alloc_tile_pool(name="psum", bufs=1, space="PSUM")
    pw = tc.alloc_tile_pool(name="psumw", bufs=2, space="PSUM")

    # ---------------- phase 0: load events + batched prep ----------------
    # per-(array, half) event/state tiles, [128 events, 32 samples]
    T = {}
    ST = {}
    for a in range(5):
        for kt in range(2):
            t_t = cp.tile([EH, S], F32, tag=f"T{a}{kt}", name="t")
            s_t = cp.tile([EH, S], I32, tag=f"S{a}{kt}", name="t")
            nc.sync.dma_start(out=t_t[:], in_=timesT_d[a, kt])
            nc.scalar.dma_start(out=s_t[:], in_=statesT_d[a, kt])
            T[a, kt] = t_t
            ST[a, kt] = s_t

    # int16 mask-threshold tiles [128 events, 32 samples]
    thr = {}
    for k in range(6):
        for kt in range(2):
            th_t = cp.tile([EH, S], F32, tag=f"thr{k}{kt}", name="t")
            (nc.sync if (k + kt) % 2 else nc.scalar).dma_start(
                out=th_t[:], in_=thr_d[k, kt])
            thr[k, kt] = th_t

    # base/weights broadcast columns (0-stride DMA from DRAM)
    wbbc = cp.tile([128, 4], F32, tag="wbbc", name="t")
    nc.vector.memset(wbbc[:], 0.0)
    nc.sync.dma_start(out=wbbc[:, 0:3], in_=bcast(weights_d[:]))
    nc.sync.dma_start(out=wbbc[:, 3:4], in_=bcast(base_d[:]))
    negw2 = cp.tile([128, 1], F32, tag="negw2", name="t")
    nc.vector.tensor_scalar(out=negw2[:], in0=wbbc[:, 2:3], scalar1=-1.0,
                            scalar2=None, op0=OP.mult)

    # consts: col0 = query-type mask (1.0 head rows), col1 = pad-column mask
    consts = cp.tile([128, 2], F32, tag="consts", name="t")
    nc.sync.dma_start(out=consts[:], in_=consts_d[:])
    qtmask = consts[:, 0:1]
    padcol = consts[:, 1:2]

    # ---- batched exponentials / state masks / weight vectors per half ----
    ew = {}     # exp tiles keyed by (name, kt)
    sm = {}
    for kt in range(2):
        # exp args -> one tile per needed exponential, [128, 32]
        def _exp(tag, src, scale, off):
            arg = sp.tile([EH, S], F32, tag=f"arg{tag}{kt}", name="t")
            nc.vector.tensor_scalar(out=arg[:], in0=src[:], scalar1=scale,
                                    scalar2=off, op0=OP.mult, op1=OP.add)
            e_t = cp.tile([EH, S], F32, tag=f"e{tag}{kt}", name="t")
            nc.scalar.activation(e_t[:], arg[:], ACTF.Exp)
            return e_t

        ew["w0", kt] = _exp("w0", T[0, kt], 1.0, -C2)       # e^{t0-C2}
        ew["c2t1", kt] = _exp("c2t1", T[1, kt], -1.0, C2)   # e^{C2-t1}
        ew["g1", kt] = _exp("g1", T[1, kt], 2.0, -2.0 * C1)  # e^{2(t1-C1)}
        ew["g2", kt] = _exp("g2", T[1, kt], 2.0, -2.0 * C2)
        ew["v21", kt] = _exp("v21", T[2, kt], 1.0, -C1)
        ew["v22", kt] = _exp("v22", T[2, kt], 1.0, -C2)
        ew["v31", kt] = _exp("v31", T[3, kt], 1.0, -C1)
        ew["v32", kt] = _exp("v32", T[3, kt], 1.0, -C2)

        for a, val, tag in ((0, 1, "s0"), (1, 1, "s1"), (2, 1, "s2"), (3, 0, "s3")):
            m = cp.tile([EH, S], F32, tag=f"{tag}{kt}", name="t")
            nc.vector.tensor_scalar(out=m[:], in0=ST[a, kt][:], scalar1=val,
                                    scalar2=None, op0=OP.is_equal)
            sm[tag, kt] = m

        # [t3 <= C1]: zeroes v3C1 entries that no blk1 query can ever select;
        # keeps sum(v3C1) bounded so D'/C' stay in fp32 range.
        m31 = cp.tile([EH, S], F32, tag=f"m31{kt}", name="t")
        nc.vector.tensor_scalar(out=m31[:], in0=T[3, kt][:], scalar1=C1,
                                scalar2=None, op0=OP.is_le)
        sm["m31", kt] = m31

    def dekker(dst, blk0, src32, tmp_tag):
        """write bf16 (hi, lo) blocks of src32 [128, S] into dst block cols
        [blk0*S:(blk0+1)*S] and [(blk0+1)*S:(blk0+2)*S]"""
        hi = dst[:, blk0 * S:(blk0 + 1) * S]
        lo = dst[:, (blk0 + 1) * S:(blk0 + 2) * S]
        nc.vector.tensor_copy(out=hi, in_=src32[:])
        tmp = sp.tile([EH, S], F32, tag=tmp_tag, name="t")
        nc.vector.tensor_copy(out=tmp[:], in_=hi)
        nc.vector.tensor_tensor(out=lo, in0=src32[:], in1=tmp[:],
                                op=OP.subtract)

    # w0 pairs (feat0 inner sum weights), [128, 2*S]: cols 2s,2s+1 = h,l
    w0pair = {}
    for kt in range(2):
        w0 = sp.tile([EH, S], F32, tag=f"w0m{kt}", name="t")
        nc.vector.tensor_tensor(out=w0[:], in0=ew["w0", kt][:], in1=sm["s0", kt][:],
                                op=OP.mult)
        pair = cp.tile([EH, 2 * S], BF16, tag=f"w0pair{kt}", name="t")
        dekker(pair, 0, w0, f"w0tmp{kt}")
        w0pair[kt] = pair

    # v2 / v3 quads [128, 4*S]: cols 4s..4s+3 = [vC1h vC1l vC2h vC2l]
    vB = {}
    vC = {}
    vCn = {}
    for kt in range(2):
        q_b = cp.tile([EH, 4 * S], BF16, tag=f"vB{kt}", name="t")
        q_c = cp.tile([EH, 4 * S], BF16, tag=f"vC{kt}", name="t")
        for ver, (e2tag, e3tag) in enumerate((("v21", "v31"), ("v22", "v32"))):
            v2 = sp.tile([EH, S], F32, tag=f"v2m{kt}{ver}", name="t")
            nc.vector.tensor_tensor(out=v2[:], in0=ew[e2tag, kt][:],
                                    in1=sm["s2", kt][:], op=OP.mult)
            dekker(q_b, 2 * ver, v2, f"dkb{kt}{ver}")
            v3 = sp.tile([EH, S], F32, tag=f"v3m{kt}{ver}", name="t")
            nc.vector.tensor_tensor(out=v3[:], in0=ew[e3tag, kt][:],
                                    in1=sm["s3", kt][:], op=OP.mult)
            if ver == 0:
                nc.vector.tensor_tensor(out=v3[:], in0=v3[:],
                                        in1=sm["m31", kt][:], op=OP.mult)
            dekker(q_c, 2 * ver, v3, f"dkc{kt}{ver}")
        # negated vC for the PSUM-side band subtract (D' - C')
        q_cn = cp.tile([EH, 4 * S], BF16, tag=f"vCn{kt}", name="t")
        nc.vector.tensor_scalar(out=q_cn[:], in0=q_c[:], scalar1=-1.0,
                                scalar2=None, op0=OP.mult)
        vB[kt] = q_b
        vC[kt] = q_c
        vCn[kt] = q_cn

    # dsh (bf16): sh_j - sh_{j-1 (wrap)} from host; stored zero-padded
    # [z z z z dsh] per sample so the E matmul lands on psum row 4.
    dsh = {}
    for kt in range(2):
        d32 = sp.tile([EH, S], F32, tag=f"dsh32{kt}", name="t")
        nc.sync.dma_start(out=d32[:], in_=dshT_d[kt])
        d = cp.tile([EH, 5 * S], BF16, tag=f"dsh{kt}", name="t")
        nc.vector.memset(d[:], 0.0)
        nc.vector.tensor_copy(out=d[:, 4 * S:5 * S], in_=d32[:])
        dsh[kt] = d

    # escol = 1 - 2*sh[255], per (sample,qt) partition row (host-computed)
    escol = cp.tile([128, 1], F32, tag="escol", name="t")
    nc.sync.dma_start(out=escol[:], in_=escol_d[:])

    # ------------- phase 1+2: per-sample What + g-vector assembly ----------
    # Processed in 8-sample blocks so phase-3 A-matmuls for block b only
    # depend on phase 1 of block b (avoids head-of-line blocking of the
    # in-order PE queue behind all 32 samples of phase 1).
    wst = cp.tile([128, 4 * S], F32, tag="wst", name="t")  # cols 4s.. = [j0h j0l j1h j1l]
    gA = {}
    for kt in range(2):
        # 5-col layout [g1h g1l g2h g2l 0]: the zero 5th column lets the A
        # matmul share psum rows 0-4 with the E matmul (E lands on row 4).
        g_t = cp.tile([EH, 5 * S], BF16, tag=f"gA{kt}", name="t")
        nc.vector.memset(g_t[:], 0.0)
        gA[kt] = g_t

    SB = 8  # phase-1/2 block size
    for blk in range(S // SB):
        for s in range(blk * SB, (blk + 1) * SB):
            t1bc = sp.tile([128, E], I16, tag="t1bc", name="t")
            nc.sync.dma_start(out=t1bc[:], in_=bcast(t1m16_d[s]))

            psw = pw.tile([128, 4], F32, tag="psw", name="t")
            mwts = []
            for ikt in range(2):
                mwt = sp.tile([128, E], BF16, tag=f"mwt{ikt}", name="t")
                nc.vector.tensor_scalar(out=mwt[:], in0=t1bc[:],
                                        scalar1=thr[5, ikt][:, s:s + 1],
                                        scalar2=None, op0=OP.is_gt)
                mwts.append(mwt)
            for jkt in range(2):
                for ikt in range(2):
                    nc.tensor.matmul(psw[:, 2 * jkt:2 * jkt + 2],
                                     mwts[ikt][:, jkt * EH:(jkt + 1) * EH],
                                     w0pair[ikt][:, s::S][:, 0:2],
                                     start=(ikt == 0), stop=(ikt == 1))
            nc.vector.tensor_copy(out=wst[:, s::S][:, 0:4], in_=psw[:])

        # phase 2 for this block: assemble gA columns blk*SB..(blk+1)*SB
        bs = slice(blk * SB, (blk + 1) * SB)
        for kt in range(2):
            wh = sp.tile([EH, SB], F32, tag=f"wh{kt}", name="t")
            # wst blocks: [j0h | j0l | j1h | j1l], each S wide
            nc.vector.tensor_tensor(
                out=wh[:], in0=wst[:, 2 * kt * S:(2 * kt + 1) * S][:, bs],
                in1=wst[:, (2 * kt + 1) * S:(2 * kt + 2) * S][:, bs],
                op=OP.add)
            nc.vector.tensor_tensor(out=wh[:], in0=wh[:],
                                    in1=ew["c2t1", kt][:, bs], op=OP.mult)
            for ver, etag in enumerate(("g1", "g2")):
                g32 = sp.tile([EH, SB], F32, tag=f"g32{kt}{ver}", name="t")
                nc.vector.tensor_tensor(out=g32[:], in0=ew[etag, kt][:, bs],
                                        in1=wh[:], op=OP.mult)
                nc.vector.tensor_tensor(out=g32[:], in0=g32[:],
                                        in1=sm["s1", kt][:, bs], op=OP.mult)
                hi = gA[kt][:, 2 * ver * S:(2 * ver + 1) * S][:, bs]
                lo = gA[kt][:, (2 * ver + 1) * S:(2 * ver + 2) * S][:, bs]
                nc.vector.tensor_copy(out=hi, in_=g32[:])
                tmp8 = sp.tile([EH, SB], F32, tag=f"dkg{kt}{ver}", name="t")
                nc.vector.tensor_copy(out=tmp8[:], in_=hi)
                nc.vector.tensor_tensor(out=lo, in0=g32[:], in1=tmp8[:],
                                        op=OP.subtract)

    # ------------- phase 3: per-sample masks + weighted sums ---------------
    # psum row groups (32-aligned, single bank): A+E@0-4 (E on row 4 via the
    # zero-padded gA/dsh lhsT columns), B@32-35, band(D'-C')@64-67.  The
    # band subtract happens in PSUM: +vC@mD then -vC@mC accumulate.  Masks
    # compare int16-quantized times so the DVE runs in its packed 16-bit
    # mode (~2x) instead of the 1x fp32 path.
    # stga32 holds all samples' evacuated results: free dim = (s, q).
    stga32 = cp.tile([128, S * Q], F32, tag="stga32", name="t")
    stage2 = cp.tile([128, 15 * 128], F32, tag="stage2", name="t")
    sd = staging_d[:]
    psums = []
    for i in range(3):
        t_ps = pp.tile([128, Q], F32, tag=f"pm{i}", name="t")
        nc.vector.memset(t_ps[:], 0.0)
        psums.append(t_ps)
    tq16big = None
    for s in range(S):
        if s % 4 == 0:
            tq16big = qp.tile([128, 4 * Q], I16, tag="tq16", name="t")
            qsrc = bass.AP(queries_d[:].tensor,
                           queries_d[:].offset + s * Q,
                           [[0, 128], [1, 4 * Q]])
            nc.sync.dma_start(out=tq16big[:], in_=qsrc)
        tq16 = tq16big[:, (s % 4) * Q:(s % 4 + 1) * Q]
        psum = psums[s % 3]
        for kt in range(2):
            masks = []
            for kind, op in ((0, OP.is_gt), (1, OP.is_gt), (2, OP.is_gt),
                             (3, OP.is_ge)):
                m = mp.tile([128, Q], BF16, tag=f"m{kind}{kt}", name="t")
                nc.vector.tensor_scalar(out=m[:], in0=tq16,
                                        scalar1=thr[kind, kt][:, s:s + 1],
                                        scalar2=None, op0=op)
                masks.append(m)
            mA, mB, mC, mD = masks
            # E only needs the 256 grid queries; head-query E (= sh_q) is
            # overwritten into stage2 directly from the host shq input.
            mE = mp.tile([128, Q // 2], BF16, tag=f"m4{kt}", name="t")
            nc.vector.tensor_scalar(out=mE[:], in0=tq16[:, Q // 2:Q],
                                    scalar1=thr[4, kt][:, s:s + 1],
                                    scalar2=None, op0=OP.is_gt)
            st = (kt == 0)
            sp_ = (kt == 1)
            nc.tensor.matmul(psum[0:5, 0:Q], gA[kt][:, s::S][:, 0:5], mA[:],
                             start=st, stop=False)
            nc.tensor.matmul(psum[0:5, Q // 2:Q], dsh[kt][:, s::S][:, 0:5],
                             mE[:], start=False, stop=sp_)
            nc.tensor.matmul(psum[32:36, 0:Q], vB[kt][:, s::S][:, 0:4], mB[:],
                             start=st, stop=sp_)
            nc.tensor.matmul(psum[64:68, 0:Q], vC[kt][:, s::S][:, 0:4], mD[:],
                             start=st, stop=False)
            nc.tensor.matmul(psum[64:68, 0:Q], vCn[kt][:, s::S][:, 0:4], mC[:],
                             start=False, stop=sp_)
        # evacuate psum -> stga32 (ACT); A+E rows 0-4 and B rows 32-35 in
        # one copy (junk rows between), band in a second.
        nc.scalar.copy(stga32[0:36, s * Q:(s + 1) * Q], psum[0:36, 0:Q])
        nc.scalar.copy(stga32[64:68, s * Q:(s + 1) * Q], psum[64:68, 0:Q])
        evac = {7: (0, 8), 15: (8, 8), 23: (16, 8), 27: (24, 4), 31: (28, 4)}
        if s in evac:
            b0, bn = evac[s]
            src_v = stga32[:].rearrange("p (s q) -> p s q", s=S)
            for g in range(3):
                nc.scalar.dma_start(
                    out=staging_d[5 * g:5 * (g + 1), b0:b0 + bn, :],
                    in_=src_v[32 * g:32 * g + 5, b0:b0 + bn, :])
            # gather this block back: staging [15 r, bn s, 512 q] ->
            # stage2 rows {32 qt + b0 + s'}, (15 r, 128 q)
            for qt in range(4):
                src_g = bass.AP(sd.tensor, sd.offset + qt * 128 + b0 * Q,
                                [[Q, bn], [S * Q, 15], [1, 128]])
                nc.scalar.dma_start(
                    out=stage2[32 * qt + b0:32 * qt + b0 + bn,
                               :].rearrange("p (r q) -> p r q", r=15),
                    in_=src_g)

    # head-query E rows: overwrite role 4 of rows 0-63 with host sh_q
    nc.sync.dma_start(out=stage2[0:64, 4 * 128:5 * 128], in_=shq_d[:])

    # ------------- phase 4: batched post-processing ------------------------
    def R(r):
        return stage2[:, r * 128:(r + 1) * 128]

    # query matrix [128 (s,qt), 128]
    tq_m = cp.tile([128, 128], F32, tag="tqm", name="t")
    nc.vector.memset(tq_m[:], 0.0)
    nc.sync.dma_start(out=tq_m[0:32, :], in_=times_d[:, 4, 1:129])
    nc.sync.dma_start(out=tq_m[32:64, 0:127], in_=times_d[:, 4, 129:256])
    nc.sync.dma_start(out=tq_m[32:64, 127:128], in_=times_d[:, 4, 255:256])
    nc.sync.dma_start(out=tq_m[64:96, :], in_=gridq_d[0])
    nc.sync.dma_start(out=tq_m[96:128, :], in_=gridq_d[1])

    def tmp(tag):
        return cp.tile([128, 128], F32, tag=tag, name="t")

    # pairwise hi+lo sums (in place into the hi slot)
    # roles: 0-3 A quads, 4 E, 5-8 B, 10-13 band=D'-C' (9, 14 junk)
    for r in (0, 2, 5, 7, 10, 12):
        nc.vector.tensor_tensor(out=R(r), in0=R(r), in1=R(r + 1), op=OP.add)
    A1, A2, B1, B2, Bd1, Bd2 = (R(r) for r in (0, 2, 5, 7, 10, 12))

    blk = cp.tile([128, 128], U8, tag="blk", name="t")
    nc.vector.tensor_scalar(out=blk[:], in0=tq_m[:], scalar1=C1, scalar2=None,
                            op0=OP.is_ge)
    biasC1 = cp.tile([128, 1], F32, tag="biasC1", name="t")
    nc.vector.memset(biasC1[:], C1)
    biasC2 = cp.tile([128, 1], F32, tag="biasC2", name="t")
    nc.vector.memset(biasC2[:], C2)
    e1 = tmp("e1")
    nc.scalar.activation(e1[:], tq_m[:], ACTF.Exp, bias=biasC1[:], scale=-1.0)
    e2 = tmp("e2")
    nc.scalar.activation(e2[:], tq_m[:], ACTF.Exp, bias=biasC2[:], scale=-1.0)

    def sel(tag, on_true, on_false):
        o = tmp(tag)
        nc.vector.select(o, blk[:], on_true, on_false)
        return o

    esel = sel("esel", e2[:], e1[:])
    Asel = sel("Asel", A2, A1)
    Bsel = sel("Bsel", B2, B1)
    Bdsel = sel("Bdsel", Bd2, Bd1)

    feat0 = tmp("feat0")
    nc.vector.tensor_tensor(out=feat0[:], in0=esel[:], in1=Asel[:], op=OP.mult)
    nc.vector.tensor_tensor(out=feat0[:], in0=feat0[:], in1=esel[:], op=OP.mult)
    feat1 = tmp("feat1")
    nc.vector.tensor_tensor(out=feat1[:], in0=esel[:], in1=Bsel[:], op=OP.mult)
    feat2 = tmp("feat2")
    nc.vector.tensor_tensor(out=feat2[:], in0=Bdsel[:], in1=esel[:], op=OP.mult)

    eff0 = tmp("eff0")
    nc.vector.tensor_scalar(out=eff0[:], in0=R(4), scalar1=-2.0, scalar2=escol[:],
                            op0=OP.mult, op1=OP.add)

    combo = tmp("combo")
    nc.vector.tensor_scalar(out=combo[:], in0=feat0[:], scalar1=wbbc[:, 0:1],
                            scalar2=None, op0=OP.mult)
    nc.vector.scalar_tensor_tensor(out=combo[:], in0=feat1[:], scalar=wbbc[:, 1:2],
                                   in1=combo[:], op0=OP.mult, op1=OP.add)
    nc.vector.scalar_tensor_tensor(out=combo[:], in0=feat2[:], scalar=negw2[:],
                                   in1=combo[:], op0=OP.mult, op1=OP.add)
    logits = tmp("logits")
    nc.vector.tensor_tensor(out=logits[:], in0=combo[:], in1=eff0[:], op=OP.mult)
    nc.vector.tensor_scalar(out=logits[:], in0=logits[:], scalar1=wbbc[:, 3:4],
                            scalar2=None, op0=OP.add)
    # zero the pad query (qt==1 rows, col 127) via the pad-column mask
    nc.vector.tensor_tensor(out=logits[:, 127:128], in0=logits[:, 127:128],
                            in1=padcol, op=OP.mult)

    hsum = cp.tile([128, 1], F32, tag="hsum", name="t")
    nc.vector.tensor_reduce(out=hsum[:], in_=logits[:], axis=AX.X, op=OP.add)
    expt = tmp("expt")
    intcol = cp.tile([128, 1], F32, tag="intcol", name="t")
    nc.scalar.activation(expt[:], logits[:], ACTF.Exp, accum_out=intcol[:])
    nc.vector.tensor_scalar(out=intcol[:], in0=intcol[:], scalar1=-RES,
                            scalar2=None, op0=OP.mult)
    qtmaski = cp.tile([128, 1], U8, tag="qtmaski", name="t")
    nc.vector.tensor_scalar(out=qtmaski[:], in0=qtmask, scalar1=0.5,
                            scalar2=None, op0=OP.is_ge)
    rowpart = cp.tile([128, 1], F32, tag="rowpart", name="t")
    nc.vector.select(rowpart[:], qtmaski[:], hsum[:], intcol[:])
    nc.sync.dma_start(out=out_d[:], in_=rowpart[:])

    for pool in (pw, pp, mp, qp, sp, cp):
        pool.release()


_NC_CACHE = []


def _get_nc():
    if not _NC_CACHE:
        _NC_CACHE.append(build_nc())
    return _NC_CACHE[0]


def _q16(x):
    """int16 compare-domain quantization (must be the single rounding
    function for every quantized value so exact f32 ties stay ties)."""
    return np.clip(np.rint(x.astype(np.float64) * QSCL + QOFF),
                   -32768, 32767).astype(np.int16)


def make_inputs_for_core(times, states, base, weights, core):
    grid = np.arange(0.0, T_MAX, RES, dtype=np.float32)
    gridq = np.stack([np.tile(grid[0:128], (S, 1)), np.tile(grid[128:256], (S, 1))])
    consts = np.ones((128, 2), np.float32)
    consts[64:128, 0] = 0.0   # qtmask: 0 for grid rows (qt 2,3 blocks)
    consts[32:64, 1] = 0.0    # pad-column mask: 0 for qt1 block
    sl = slice(core * S, (core + 1) * S)
    t = np.ascontiguousarray(times[sl]).astype(np.float32)
    st = np.ascontiguousarray(states[sl]).astype(np.int32)
    queries_f = np.concatenate(
        [t[:, 4, 1:256], t[:, 4, 255:256], np.tile(grid, (S, 1))],
        axis=1).astype(np.float32)
    tolf = np.float32(TOL)
    # [6, S, 2, EH] -> transpose to [6, 2, EH, S] (event-major)
    thr = np.stack([
        _q16((t[:, 1] + tolf).astype(np.float32)),
        _q16((t[:, 2] + tolf).astype(np.float32)),
        _q16((t[:, 3] + tolf).astype(np.float32)),
        _q16(t[:, 3]),
        _q16(t[:, 4]),
        _q16(t[:, 0]),
    ]).astype(np.float32).reshape(6, S, 2, EH).transpose(0, 2, 3, 1)
    timesT = t.reshape(S, 5, 2, EH).transpose(1, 2, 3, 0)
    statesT = st.reshape(S, 5, 2, EH).transpose(1, 2, 3, 0)
    sh = st[:, 4, :]
    dsh = (sh - np.roll(sh, 1, axis=1)).astype(np.float32)   # [S, E]
    dshT = dsh.reshape(S, 2, EH).transpose(1, 2, 0)
    es_g = (1 - 2 * sh[:, 255]).astype(np.float32)
    escol = np.concatenate([np.ones(64, np.float32), np.tile(es_g, 2)])
    t1m16 = _q16((t[:, 1] - tolf).astype(np.float32))
    return {
        "times": t,
        "timesT": np.ascontiguousarray(timesT),
        "statesT": np.ascontiguousarray(statesT),
        "base": np.asarray(base, np.float32),
        "weights": np.asarray(weights, np.float32),
        "queries": np.ascontiguousarray(_q16(queries_f)),
        "thr": np.ascontiguousarray(thr),
        "t1m16": np.ascontiguousarray(t1m16),
        "dshT": np.ascontiguousarray(dshT),
        "escol": escol,
        "shq": np.ascontiguousarray(
            np.concatenate([sh[:, 0:128], sh[:, 128:256]]).astype(np.float32)),
        "gridq": np.ascontiguousarray(gridq).astype(np.float32),
        "consts": consts,
    }


def kernel(times, states, base, weights):
    from concourse.bass_utils import run_bass_kernel_spmd

    times = np.asarray(times, np.float32)
    states = np.asarray(states, np.int32)
    nc = _get_nc()
    in_maps = [make_inputs_for_core(times, states, base, weights, c)
               for c in range(NCORES)]
    res = run_bass_kernel_spmd(nc, in_maps, list(range(NCORES)))
    parts = np.stack([np.asarray(res.results[c]["out"]) for c in range(NCORES)])
    total = np.sum(parts.astype(np.float32), dtype=np.float32)
    return np.array([total], dtype=np.float32)


def run_traced(times, states, base, weights):
    """Profiled run; returns HW exec time in ns (or None if tracing off)."""
    from concourse.bass_utils import run_bass_kernel_spmd

    times = np.asarray(times, np.float32)
    states = np.asarray(states, np.int32)
    nc = _get_nc()
    in_maps = [make_inputs_for_core(times, states, base, weights, c)
               for c in range(NCORES)]
    res = run_bass_kernel_spmd(nc, in_maps, list(range(NCORES)), trace=True)
    return res.exec_time_ns
